# revision 22
# baseline (speedup 1.0000x reference)
"""MoE block (top-2 routed 3x3 conv experts) Trainium2 Bass kernel.

Strategy: data-parallel over batch, 2 samples per core on 8 cores.
Since the conv is linear in the kernel, combine the top-2 expert kernels
with the routing probabilities first (w_comb = sum_e p_e * W_e), then do a
single 3x3 SAME conv per sample, plus bias + residual.

Conv-as-matmul layout: x arrives host-padded to [130x130] per channel
(zero borders) and lives in SBUF flat, partitions 0-63 = channels,
partitions 64-127 = the same channels shifted by +2 elements. A single
[128, 4x128] rhs read then provides taps (dy,-1) on top and (dy,+1) on
the bottom half, so the six dx=+-1 taps are three K=128 matmuls; the
three dx=0 taps are K=128 matmuls with zero weights on the bottom half
(K=64 matmuls measure ~1.8x slower). All conv matmuls run in float32r
(single-pass fp32 mode, ~235ns per [128,64]x[128,512] MM).

DMA: a per-engine DGE lane moves ~150-190 GB/s and transfers on one
lane serialize, so: SP lane = top halves + half of the expert weights +
even out tiles; ACT lane = bottom halves (rows 64-127 first, feeding
pooled) + odd out tiles; gpsimd SWDGE = tiny gate weights + the other
half of the expert weights. Conv tiles dep on the whole XX tile, so
lane queues are kept as short as possible before the last x chunk.
Pooled GAP partials: top chunks 0,1 on ACT (ACTIVATE+accum interleaves
with its DMA queue), bottom chunks 2,3 on DVE; the gate matmul uses a
[wg1; wg1]-stacked lhsT to fold the cross-half sum. Gate math runs on
DVE except the softmax exp (ACT, ordered before the late ACT DMAs).
"""
import numpy as np
from contextlib import ExitStack

import concourse.bass as bass
import concourse.tile as tile
from concourse import bacc, mybir
from concourse.bass_utils import run_bass_kernel_spmd
from concourse.tile import add_dep_helper

F32 = mybir.dt.float32
F32R = mybir.dt.float32r
AX = mybir.AxisListType
OP = mybir.AluOpType
ACTF = mybir.ActivationFunctionType

B, C, H, W, E, GH = 16, 64, 128, 128, 8, 16
NCORES = 8
SPB = B // NCORES          # samples per core
HP, WP = H + 2, W + 2      # 130
FLAT = HP * WP             # 16900
NT = H // 4                # 32 conv tiles of 4 rows each
GATE_SPLIT = 18            # emit sample-1 gate work after this many s0 conv tiles
# x-load chunk boundaries in padded rows: [0,33) [33,65) [65,97) [97,130)
CHUNK_ROWS = [(0, 33), (33, 65), (65, 97), (97, 130)]

_cache = {}


def _emit_sample_loads(nc, pools, s, xs_ap):
    """Input DMAs + pooled partial sums for sample s (all contiguous)."""
    f = pools
    XX = f["xx"].tile([128, FLAT], F32R, tag="XX", name=f"XX{s}")

    def top(k):
        c0, c1 = CHUNK_ROWS[k][0] * WP, CHUNK_ROWS[k][1] * WP
        nc.sync.dma_start(XX[0:64, c0:c1], xs_ap[s, :, c0:c1])

    def bot(k):
        c0, c1 = CHUNK_ROWS[k][0] * WP, CHUNK_ROWS[k][1] * WP
        d0 = max(c0 - 2, 0)
        d1 = FLAT if k == 3 else c1 - 2
        return nc.scalar.dma_start(XX[64:128, d0:d1], xs_ap[s, :, d0 + 2 : d1 + 2])

    top(0)
    top(1)
    if s == 0:
        nc.sync.dma_start(f["wpsA_sb"][:], f["wpsA_d"][:])
    top(2)
    top(3)

    part = f["gate"].tile([128, 2], F32, tag="part", name=f"part{s}")
    bot(2)
    bot(3)
    # ACT-lane compute: pooled partials of top chunks 0,1 (zeros included)
    scrA = f["scratch"].tile([64, 33 * WP], F32, tag="scrA", name=f"scrA{s}")
    for k in (0, 1):
        c0, c1 = CHUNK_ROWS[k][0] * WP, CHUNK_ROWS[k][1] * WP
        nc.scalar.activation(
            scrA[:, 0 : c1 - c0],
            XX[0:64, c0:c1].bitcast(F32),
            ACTF.Copy,
            accum_out=part[0:64, k : k + 1],
        )
    bot_dmas = [bot(0), bot(1)]
    # DVE: pooled partials of bottom chunks 2,3 (partitions 64-127)
    scrB = f["scratch"].tile([128, 33 * WP], F32, tag="scrB", name=f"scrB{s}")
    for k in (2, 3):
        c0, c1 = CHUNK_ROWS[k][0] * WP - 2, CHUNK_ROWS[k][1] * WP - 2
        nc.vector.tensor_scalar(
            scrB[64:128, 0 : c1 - c0],
            XX[64:128, c0:c1].bitcast(F32),
            0.0,
            0.0,
            OP.add,
            OP.add,
            accum_out=part[64:128, k - 2 : k - 1],
        )
    pooled = f["gate"].tile([128, 1], F32, tag="pooled", name=f"pooled{s}")
    nc.vector.tensor_reduce(pooled, part[:], axis=AX.X, op=OP.add)
    return XX, pooled, bot_dmas


def _emit_sample_gate(nc, pools, s, pooled, consts):
    """Gate MLP + softmax + top-2 + combined weights/bias for one sample.

    Uses exp-without-max-sub (logits are small) and folds the top-2 mask
    and renormalization:  w8 = (u>=m2)*u / (sum((u>=m2)*u) + sum(u)*1e-8)
    which equals the reference's normalized-probs formula exactly.
    Returns (wcombr, b_comb, exp_inst).
    """
    f = pools
    g = f["gate"]
    wg1x2_sb, bg1_sb, wg2_sb, bg2_sb, bexp_sb, wpsA_sb, wpsB_sb, ones = consts
    n = lambda base: f"{base}{s}"

    h_ps = f["gpsum"].tile([GH, 1], F32, tag="gps", name=n("h_ps"))
    nc.tensor.matmul(h_ps[:], lhsT=wg1x2_sb[:], rhs=pooled[:], start=True, stop=True)
    # h = relu(pooled_sum/(H*W) @ wg1 + bg1)
    h1 = g.tile([GH, 1], F32, tag="h1", name=n("h1"))
    nc.vector.scalar_tensor_tensor(
        h1[:], h_ps[:], 1.0 / (H * W), bg1_sb[:], op0=OP.mult, op1=OP.add
    )
    h_sb = g.tile([GH, 1], F32, tag="h_sb", name=n("h_sb"))
    nc.vector.tensor_scalar_max(h_sb[:], h1[:], 0.0)

    lg_ps = f["gpsum"].tile([1, E], F32, tag="gps", name=n("lg_ps"))
    nc.tensor.matmul(lg_ps[:], lhsT=h_sb[:], rhs=wg2_sb[:], start=True, stop=True)
    lg = g.tile([1, E], F32, tag="lg", name=n("lg"))
    nc.vector.tensor_add(lg[:], lg_ps[:], bg2_sb[:])

    # u = exp(logits) (unnormalized softmax; |logits| is tiny, no max-sub)
    u = g.tile([1, E], F32, tag="u", name=n("u"))
    exp_inst = nc.scalar.activation(u[:], lg[:], ACTF.Exp)
    usum = g.tile([1, 1], F32, tag="usum", name=n("usum"))
    nc.vector.tensor_reduce(usum[:], u[:], axis=AX.X, op=OP.add)
    # top-2: pm = (u < max)*u (valid since u>0), m2 = 2nd max, spv = (u>=m2)*u
    m1p = g.tile([1, 1], F32, tag="m1p", name=n("m1p"))
    nc.vector.tensor_reduce(m1p[:], u[:], axis=AX.X, op=OP.max)
    pm = g.tile([1, E], F32, tag="pm", name=n("pm"))
    nc.vector.scalar_tensor_tensor(pm[:], u[:], m1p[:], u[:], op0=OP.is_lt, op1=OP.mult)
    m2 = g.tile([1, 1], F32, tag="m2", name=n("m2"))
    nc.vector.tensor_reduce(m2[:], pm[:], axis=AX.X, op=OP.max)
    spv = g.tile([1, E], F32, tag="spv", name=n("spv"))
    nc.vector.scalar_tensor_tensor(spv[:], u[:], m2[:], u[:], op0=OP.is_ge, op1=OP.mult)
    dsum = g.tile([1, 1], F32, tag="dsum", name=n("dsum"))
    nc.vector.tensor_reduce(dsum[:], spv[:], axis=AX.X, op=OP.add)
    dd = g.tile([1, 1], F32, tag="dd", name=n("dd"))
    nc.vector.scalar_tensor_tensor(dd[:], usum[:], 1e-8, dsum[:], op0=OP.mult, op1=OP.add)
    rr = g.tile([1, 1], F32, tag="rr", name=n("rr"))
    nc.vector.reciprocal(rr[:], dd[:])
    w8 = g.tile([1, E], F32, tag="w8", name=n("w8"))
    nc.vector.tensor_scalar_mul(w8[:], spv[:], rr[:])

    # broadcast w8 down all 128 partitions: [128, E] = ones[1,128]^T @ w8[1,E]
    wb_ps = f["gpsum"].tile([128, E], F32, tag="gps", name=n("wb_ps"))
    nc.tensor.matmul(wb_ps[:], lhsT=ones[:], rhs=w8[:], start=True, stop=True)
    wb128 = g.tile([128, E], F32, tag="wb128", name=n("wb128"))
    nc.vector.tensor_copy(wb128[:], wb_ps[:])

    # combined bias path (off critical path): b_comb = b_exp^T @ w8^T
    w8c_ps = f["gpsum"].tile([E, 1], F32, tag="gps", name=n("w8c_ps"))
    nc.tensor.matmul(w8c_ps[:], lhsT=w8[:], rhs=ones[:, 0:1], start=True, stop=True)
    w8col = g.tile([E, 1], F32, tag="w8col", name=n("w8col"))
    nc.vector.tensor_copy(w8col[:], w8c_ps[:])
    bc_ps = f["gpsum"].tile([C, 1], F32, tag="gps", name=n("bc_ps"))
    nc.tensor.matmul(bc_ps[:], lhsT=bexp_sb[:], rhs=w8col[:], start=True, stop=True)
    b_comb = g.tile([C, 1], F32, tag="b_comb", name=n("b_comb"))
    nc.vector.tensor_copy(b_comb[:], bc_ps[:])

    # combined conv weights: one fused MAC chain over [128, 6, C]
    # (slots 0-2 = paired dx taps, 3-5 = dx=0 taps w/ zero bottom rows)
    wcomb = f["wcomb"].tile([128, 6, C], F32, tag="wcomb", name=n("wcomb"))
    nc.vector.tensor_scalar_mul(wcomb[:], wpsA_sb[:, 0], wb128[:, 0:1])
    for e in range(1, E):
        src_w = wpsA_sb[:, e] if e < 4 else wpsB_sb[:, e - 4]
        nc.vector.scalar_tensor_tensor(
            wcomb[:], src_w, wb128[:, e : e + 1], wcomb[:],
            op0=OP.mult, op1=OP.add,
        )
    wcombr = f["wcomb"].tile([128, 6, C], F32R, tag="wcombr", name=n("wcombr"))
    nc.vector.tensor_copy(wcombr[:], wcomb[:])
    return wcombr, b_comb, exp_inst


def _emit_conv_tiles(nc, pools, s, XX, wcombr, b_comb, out_ap, t_range):
    """Conv tiles (4 output rows each) for sample s."""
    f = pools
    XX3 = XX[:, 0:FLAT].rearrange("p (r c) -> p r c", c=WP)
    for t in t_range:
        ps = f["cpsum"].tile([C, 4 * W], F32, tag="cps", name=f"cps{s}_{t}")
        r0 = 4 * t
        for dyi in range(3):
            nc.tensor.matmul(
                ps[:],
                lhsT=wcombr[:, dyi, :],
                rhs=XX3[:, r0 + dyi : r0 + dyi + 4, 0:128],
                start=(dyi == 0),
                stop=False,
            )
        for dyi in range(3):
            nc.tensor.matmul(
                ps[:],
                lhsT=wcombr[:, 3 + dyi, :],
                rhs=XX3[:, r0 + dyi : r0 + dyi + 4, 1:129],
                start=False,
                stop=(dyi == 2),
            )
        out_sb = f["stage"].tile([C, 4, W], F32, tag="stage", name=f"ost{s}_{t}")
        nc.vector.scalar_tensor_tensor(
            out_sb[:],
            ps[:].rearrange("p (a b) -> p a b", b=W),
            b_comb[:],
            XX3[0:64, r0 + 1 : r0 + 5, 1:129].bitcast(F32),
            op0=OP.add,
            op1=OP.add,
        )
        eng = nc.sync if t % 2 == 0 else nc.scalar
        eng.dma_start(out_ap[s, :, r0 : r0 + 4, :], out_sb[:])


def build_program():
    if "nc" in _cache:
        return _cache["nc"]
    nc = bacc.Bacc("TRN2", target_bir_lowering=False, debug=False, enable_asserts=False)
    xs_ap = nc.dram_tensor("xs", [SPB, C, FLAT + 2], F32R, kind="ExternalInput").ap()
    wpsA_d = nc.dram_tensor("wpsA", [128, E // 2, 6, C], F32, kind="ExternalInput").ap()
    wpsB_d = nc.dram_tensor("wpsB", [128, E // 2, 6, C], F32, kind="ExternalInput").ap()
    wg1_d = nc.dram_tensor("wg1", [128, GH], F32, kind="ExternalInput").ap()
    bg1_d = nc.dram_tensor("bg1", [GH, 1], F32, kind="ExternalInput").ap()
    wg2_d = nc.dram_tensor("wg2", [GH, E], F32, kind="ExternalInput").ap()
    bg2_d = nc.dram_tensor("bg2", [1, E], F32, kind="ExternalInput").ap()
    bexp_d = nc.dram_tensor("b_exp", [E, C], F32, kind="ExternalInput").ap()
    out_ap = nc.dram_tensor("out", [SPB, C, H, W], F32, kind="ExternalOutput").ap()

    with tile.TileContext(nc) as tc, ExitStack() as ctx:
        pools = {
            "const": ctx.enter_context(tc.tile_pool(name="const", bufs=1)),
            "xx": ctx.enter_context(tc.tile_pool(name="xx", bufs=SPB)),
            "gate": ctx.enter_context(tc.tile_pool(name="gate", bufs=2)),
            "wcomb": ctx.enter_context(tc.tile_pool(name="wcomb", bufs=2)),
            "stage": ctx.enter_context(tc.tile_pool(name="stage", bufs=6)),
            "scratch": ctx.enter_context(tc.tile_pool(name="scratch", bufs=1)),
            "gpsum": ctx.enter_context(tc.tile_pool(name="gpsum", bufs=2, space="PSUM")),
            "cpsum": ctx.enter_context(tc.tile_pool(name="cpsum", bufs=6, space="PSUM")),
        }
        cp = pools["const"]
        ones = cp.tile([1, 128], F32)
        nc.gpsimd.memset(ones[:], 1.0)
        # prewarm the ACT exp table before the ACT lane fills with DMAs
        warm = cp.tile([1, 1], F32)
        nc.scalar.activation(warm[:], ones[:, 0:1], ACTF.Exp)
        # tiny gate weights + half the expert weights on the gpsimd SWDGE lane
        wg1x2_sb = cp.tile([128, GH], F32)
        nc.gpsimd.dma_start(wg1x2_sb[:], wg1_d[:])
        bg1_sb = cp.tile([GH, 1], F32)
        nc.gpsimd.dma_start(bg1_sb[:], bg1_d[:])
        wg2_sb = cp.tile([GH, E], F32)
        nc.gpsimd.dma_start(wg2_sb[:], wg2_d[:])
        bg2_sb = cp.tile([1, E], F32)
        nc.gpsimd.dma_start(bg2_sb[:], bg2_d[:])
        bexp_sb = cp.tile([E, C], F32)
        nc.gpsimd.dma_start(bexp_sb[:], bexp_d[:])
        wpsA_sb = cp.tile([128, E // 2, 6, C], F32)
        wpsB_sb = cp.tile([128, E // 2, 6, C], F32)
        nc.gpsimd.dma_start(wpsB_sb[:], wpsB_d[:])
        pools["wpsA_sb"] = wpsA_sb
        pools["wpsA_d"] = wpsA_d

        XX0, pooled0, bots0 = _emit_sample_loads(nc, pools, 0, xs_ap)
        consts = (wg1x2_sb, bg1_sb, wg2_sb, bg2_sb, bexp_sb, wpsA_sb, wpsB_sb, ones)

        g0 = _emit_sample_gate(nc, pools, 0, pooled0, consts)
        # keep s0's exp ahead of the late bottom chunks on the ACT lane
        add_dep_helper(bots0[0].ins, g0[2].ins, sync=False,
                       reason="s0 late bottom DMAs after s0 softmax exp")
        XX1, pooled1, bots1 = _emit_sample_loads(nc, pools, 1, xs_ap)

        _emit_conv_tiles(nc, pools, 0, XX0, *g0[:2], out_ap, range(0, GATE_SPLIT))
        g1 = _emit_sample_gate(nc, pools, 1, pooled1, consts)
        add_dep_helper(bots1[0].ins, g1[2].ins, sync=False,
                       reason="s1 late bottom DMAs after s1 softmax exp")
        _emit_conv_tiles(nc, pools, 0, XX0, *g0[:2], out_ap, range(GATE_SPLIT, NT))
        _emit_conv_tiles(nc, pools, 1, XX1, *g1[:2], out_ap, range(0, NT))

    nc.compile()
    _cache["nc"] = nc
    return nc


def _round_fp32r(a):
    """Round fp32 array to the fp32r grid (RNE to 11-bit mantissa, low 12
    bits of the fp32 word zeroed) — what the PE consumes in fp32r mode."""
    u = np.ascontiguousarray(a, dtype=np.float32).view(np.uint32)
    r = (u + np.uint32(0x7FF) + ((u >> np.uint32(12)) & np.uint32(1))) & np.uint32(
        0xFFFFF000
    )
    return r.view(np.float32)


def host_prep(x, wg1, bg1, wg2, bg2, w_exp, b_exp):
    """Host-side layout prep + per-core sharding. Returns in_maps list."""
    x = _round_fp32r(np.asarray(x, dtype=np.float32))
    xpad = np.zeros((B, C, FLAT + 2), dtype=np.float32)
    xpad[:, :, :FLAT].reshape(B, C, HP, WP)[:, :, 1 : H + 1, 1 : W + 1] = x
    wg1 = np.asarray(wg1, dtype=np.float32)
    bg1 = np.asarray(bg1, dtype=np.float32).reshape(GH, 1)
    wg2 = np.asarray(wg2, dtype=np.float32)
    bg2 = np.asarray(bg2, dtype=np.float32).reshape(1, E)
    w_exp = np.asarray(w_exp, dtype=np.float32)
    b_exp = np.asarray(b_exp, dtype=np.float32)

    # w_exp [E, O, I, KH, KW] -> wt [I, E, KH, KW, O]
    wt = np.transpose(w_exp, (2, 0, 3, 4, 1))
    # paired taps: top partitions = dx=-1, bottom = dx=+1
    wpair = np.concatenate([wt[:, :, :, 0, :], wt[:, :, :, 2, :]], axis=0)
    # single taps: dx=0 on top, zeros on bottom
    wsing = np.concatenate([wt[:, :, :, 1, :], np.zeros_like(wt[:, :, :, 1, :])], axis=0)
    # merged [128, E, 6, O]: slots 0-2 pairs, 3-5 singles
    wps = np.concatenate([wpair, wsing], axis=2)

    shared = {
        "wpsA": np.ascontiguousarray(wps[:, 0:4]),
        "wpsB": np.ascontiguousarray(wps[:, 4:8]),
        "wg1": np.ascontiguousarray(np.concatenate([wg1, wg1], axis=0)),
        "bg1": np.ascontiguousarray(bg1),
        "wg2": np.ascontiguousarray(wg2),
        "bg2": np.ascontiguousarray(bg2),
        "b_exp": np.ascontiguousarray(b_exp),
    }
    return [
        {"xs": np.ascontiguousarray(xpad[SPB * k : SPB * (k + 1)]), **shared}
        for k in range(NCORES)
    ]


def kernel(x, wg1, bg1, wg2, bg2, w_exp, b_exp):
    nc = build_program()
    in_maps = host_prep(x, wg1, bg1, wg2, bg2, w_exp, b_exp)
    res = run_bass_kernel_spmd(nc, in_maps, list(range(NCORES)))
    return np.concatenate([res.results[k]["out"] for k in range(NCORES)], axis=0)


# revision 23
# speedup vs baseline: 1.1112x; 1.1112x over previous
"""MoE block (top-2 routed 3x3 conv experts) Trainium2 Bass kernel.

Strategy: data-parallel over batch, 2 samples per core on 8 cores.
Since the conv is linear in the kernel, combine the top-2 expert kernels
with the routing probabilities first (w_comb = sum_e p_e * W_e), then do a
single 3x3 SAME conv per sample, plus bias + residual.

Conv-as-matmul layout: x is stored zero-padded [130x130] per channel in
SBUF, flat, with partitions 0-63 = channels and partitions 64-127 = the
same channels shifted by +2 pixels. A single [128, 4x128] rhs read then
provides taps (dy,-1) on top and (dy,+1) on the bottom half, so the six
dx=+-1 taps are three K=128 matmuls; the three dx=0 taps are K=128
matmuls with zero weights on the bottom half (K=64 matmuls measure ~1.8x
slower). All conv matmuls run in float32r (single-pass fp32 mode,
~235ns per [128,64]x[128,512] MM).

DMA: each engine's DGE lane moves ~150-190 GB/s and its transfers
serialize, so: SP lane = top halves + expert weights A + even out
tiles; ACT lane = bottom halves (rows 64-127 first, feeding pooled) +
odd out tiles; gpsimd SWDGE = border memsets first, then tiny gate
weights + expert weights B. Conv tiles dep on the whole XX tile, so
lanes carry as little as possible before the last x chunk. Pooled GAP
partials: top chunks 0,1 via ACT ACTIVATE+accum (interleaves with its
DMA queue), bottom chunks 2,3 on DVE; the gate matmul uses a
[wg1; wg1]-stacked lhsT to fold the cross-half sum. Gate math runs on
DVE except the softmax exp (ACT, ordered before the late ACT DMAs).
"""
import numpy as np
from contextlib import ExitStack

import concourse.bass as bass
import concourse.tile as tile
from concourse import bacc, mybir
from concourse.bass_utils import run_bass_kernel_spmd
from concourse.tile import add_dep_helper

F32 = mybir.dt.float32
F32R = mybir.dt.float32r
AX = mybir.AxisListType
OP = mybir.AluOpType
ACTF = mybir.ActivationFunctionType

B, C, H, W, E, GH = 16, 64, 128, 128, 8, 16
NCORES = 8
SPB = B // NCORES          # samples per core
HP, WP = H + 2, W + 2      # 130
FLAT = HP * WP             # 16900
NT = H // 4                # 32 conv tiles of 4 rows each
RB = 32                    # x-load chunk rows
GATE_SPLIT = 18            # emit sample-1 gate work after this many s0 conv tiles

_cache = {}


def _emit_borders(nc, XX):
    """Zero the padded borders (disjoint from the DMA-written interiors)."""
    nc.gpsimd.memset(XX[0:64, 0:130].bitcast(F32), 0.0)
    mid_top = XX[0:64, 130:16770].rearrange("p (r c) -> p r c", c=WP)
    nc.gpsimd.memset(mid_top[:, :, 0:1].bitcast(F32), 0.0)
    nc.gpsimd.memset(mid_top[:, :, 129:130].bitcast(F32), 0.0)
    nc.gpsimd.memset(XX[0:64, 16770:16900].bitcast(F32), 0.0)
    nc.gpsimd.memset(XX[64:128, 0:129].bitcast(F32), 0.0)
    mid_bot = XX[64:128, 257:16767].rearrange("p (r c) -> p r c", c=WP)
    nc.gpsimd.memset(mid_bot[:, :, 0:2].bitcast(F32), 0.0)
    nc.gpsimd.memset(XX[64:128, 16767:16900].bitcast(F32), 0.0)


def _emit_sample_loads(nc, pools, s, XX, xs_ap, mid_sp=None):
    """Input DMAs + pooled partial sums for sample s.

    SP lane: top chunks 0,1, [mid_sp()], top chunks 2,3.
    ACT lane: bottom chunks 2,3 (rows 64-127, pooled inputs), two
    ACTIVATE+accum pooled sums over top chunks 0,1, then bottom chunks
    0,1 (ordered after the sample's softmax exp by the caller).
    DVE sums bottom chunks 2,3 (partitions 64-127).
    """
    f = pools
    top_int = XX[0:64, 131:16771].rearrange("p (r c) -> p r c", c=WP)
    bot_int = XX[64:128, 129:16769].rearrange("p (r c) -> p r c", c=WP)

    def top(k):
        nc.sync.dma_start(
            top_int[:, RB * k : RB * (k + 1), 0:128],
            xs_ap[s, :, RB * k : RB * (k + 1), :],
        )

    def bot(k):
        return nc.scalar.dma_start(
            bot_int[:, RB * k : RB * (k + 1), 0:128],
            xs_ap[s, :, RB * k : RB * (k + 1), :],
        )

    top(0)
    top(1)
    if mid_sp is not None:
        mid_sp()
    top(2)
    top(3)

    part = f["gate"].tile([128, 2], F32, tag="part", name=f"part{s}")
    bot(2)
    bot(3)
    # ACT-lane compute: pooled partials of top chunks 0,1
    scrA = f["scratch"].tile([64, RB, 128], F32, tag="scrA", name=f"scrA{s}")
    for k in (0, 1):
        nc.scalar.activation(
            scrA[:],
            top_int[:, RB * k : RB * (k + 1), 0:128].bitcast(F32),
            ACTF.Copy,
            accum_out=part[0:64, k : k + 1],
        )
    bot_dmas = [bot(0), bot(1)]
    # DVE: pooled partials of bottom chunks 2,3 (partitions 64-127)
    scrB = f["scratch"].tile([128, RB, 128], F32, tag="scrB", name=f"scrB{s}")
    for k in (2, 3):
        nc.vector.tensor_scalar(
            scrB[64:128],
            bot_int[:, RB * k : RB * (k + 1), 0:128].bitcast(F32),
            0.0,
            0.0,
            OP.add,
            OP.add,
            accum_out=part[64:128, k - 2 : k - 1],
        )
    pooled = f["gate"].tile([128, 1], F32, tag="pooled", name=f"pooled{s}")
    nc.vector.tensor_reduce(pooled, part[:], axis=AX.X, op=OP.add)
    return pooled, bot_dmas


def _emit_sample_gate(nc, pools, s, pooled, consts):
    """Gate MLP + softmax + top-2 + combined weights/bias for one sample.

    Uses exp-without-max-sub (logits are small) and folds the top-2 mask
    and renormalization:  w8 = (u>=m2)*u / (sum((u>=m2)*u) + sum(u)*1e-8)
    which equals the reference's normalized-probs formula exactly.
    Returns (wcombr, b_comb, exp_inst).
    """
    f = pools
    g = f["gate"]
    wg1x2_sb, bg1_sb, wg2_sb, bg2_sb, bexp_sb, wpsA_sb, wpsB_sb, ones = consts
    n = lambda base: f"{base}{s}"

    h_ps = f["gpsum"].tile([GH, 1], F32, tag="gps", name=n("h_ps"))
    nc.tensor.matmul(h_ps[:], lhsT=wg1x2_sb[:], rhs=pooled[:], start=True, stop=True)
    # h = relu(pooled_sum/(H*W) @ wg1 + bg1)
    h1 = g.tile([GH, 1], F32, tag="h1", name=n("h1"))
    nc.vector.scalar_tensor_tensor(
        h1[:], h_ps[:], 1.0 / (H * W), bg1_sb[:], op0=OP.mult, op1=OP.add
    )
    h_sb = g.tile([GH, 1], F32, tag="h_sb", name=n("h_sb"))
    nc.vector.tensor_scalar_max(h_sb[:], h1[:], 0.0)

    lg_ps = f["gpsum"].tile([1, E], F32, tag="gps", name=n("lg_ps"))
    nc.tensor.matmul(lg_ps[:], lhsT=h_sb[:], rhs=wg2_sb[:], start=True, stop=True)
    lg = g.tile([1, E], F32, tag="lg", name=n("lg"))
    nc.vector.tensor_add(lg[:], lg_ps[:], bg2_sb[:])

    # u = exp(logits) (unnormalized softmax; |logits| is tiny, no max-sub)
    u = g.tile([1, E], F32, tag="u", name=n("u"))
    exp_inst = nc.scalar.activation(u[:], lg[:], ACTF.Exp)
    usum = g.tile([1, 1], F32, tag="usum", name=n("usum"))
    nc.vector.tensor_reduce(usum[:], u[:], axis=AX.X, op=OP.add)
    # top-2: pm = (u < max)*u (valid since u>0), m2 = 2nd max, spv = (u>=m2)*u
    m1p = g.tile([1, 1], F32, tag="m1p", name=n("m1p"))
    nc.vector.tensor_reduce(m1p[:], u[:], axis=AX.X, op=OP.max)
    pm = g.tile([1, E], F32, tag="pm", name=n("pm"))
    nc.vector.scalar_tensor_tensor(pm[:], u[:], m1p[:], u[:], op0=OP.is_lt, op1=OP.mult)
    m2 = g.tile([1, 1], F32, tag="m2", name=n("m2"))
    nc.vector.tensor_reduce(m2[:], pm[:], axis=AX.X, op=OP.max)
    spv = g.tile([1, E], F32, tag="spv", name=n("spv"))
    nc.vector.scalar_tensor_tensor(spv[:], u[:], m2[:], u[:], op0=OP.is_ge, op1=OP.mult)
    dsum = g.tile([1, 1], F32, tag="dsum", name=n("dsum"))
    nc.vector.tensor_reduce(dsum[:], spv[:], axis=AX.X, op=OP.add)
    dd = g.tile([1, 1], F32, tag="dd", name=n("dd"))
    nc.vector.scalar_tensor_tensor(dd[:], usum[:], 1e-8, dsum[:], op0=OP.mult, op1=OP.add)
    rr = g.tile([1, 1], F32, tag="rr", name=n("rr"))
    nc.vector.reciprocal(rr[:], dd[:])
    w8 = g.tile([1, E], F32, tag="w8", name=n("w8"))
    nc.vector.tensor_scalar_mul(w8[:], spv[:], rr[:])

    # broadcast w8 down all 128 partitions: [128, E] = ones[1,128]^T @ w8[1,E]
    wb_ps = f["gpsum"].tile([128, E], F32, tag="gps", name=n("wb_ps"))
    nc.tensor.matmul(wb_ps[:], lhsT=ones[:], rhs=w8[:], start=True, stop=True)
    wb128 = g.tile([128, E], F32, tag="wb128", name=n("wb128"))
    nc.vector.tensor_copy(wb128[:], wb_ps[:])

    # combined bias path (off critical path): b_comb = b_exp^T @ w8^T
    w8c_ps = f["gpsum"].tile([E, 1], F32, tag="gps", name=n("w8c_ps"))
    nc.tensor.matmul(w8c_ps[:], lhsT=w8[:], rhs=ones[:, 0:1], start=True, stop=True)
    w8col = g.tile([E, 1], F32, tag="w8col", name=n("w8col"))
    nc.vector.tensor_copy(w8col[:], w8c_ps[:])
    bc_ps = f["gpsum"].tile([C, 1], F32, tag="gps", name=n("bc_ps"))
    nc.tensor.matmul(bc_ps[:], lhsT=bexp_sb[:], rhs=w8col[:], start=True, stop=True)
    b_comb = g.tile([C, 1], F32, tag="b_comb", name=n("b_comb"))
    nc.vector.tensor_copy(b_comb[:], bc_ps[:])

    # combined conv weights: one fused MAC chain over [128, 6, C]
    # (slots 0-2 = paired dx taps, 3-5 = dx=0 taps w/ zero bottom rows)
    wcomb = f["wcomb"].tile([128, 6, C], F32, tag="wcomb", name=n("wcomb"))
    nc.vector.tensor_scalar_mul(wcomb[:], wpsA_sb[:, 0], wb128[:, 0:1])
    for e in range(1, E):
        src_w = wpsA_sb[:, e] if e < 4 else wpsB_sb[:, e - 4]
        nc.vector.scalar_tensor_tensor(
            wcomb[:], src_w, wb128[:, e : e + 1], wcomb[:],
            op0=OP.mult, op1=OP.add,
        )
    wcombr = f["wcomb"].tile([128, 6, C], F32R, tag="wcombr", name=n("wcombr"))
    nc.vector.tensor_copy(wcombr[:], wcomb[:])
    return wcombr, b_comb, exp_inst


def _emit_conv_tiles(nc, pools, s, XX, wcombr, b_comb, out_ap, t_range):
    """Conv tiles (4 output rows each) for sample s."""
    f = pools
    XX3 = XX[:, 0:FLAT].rearrange("p (r c) -> p r c", c=WP)
    for t in t_range:
        ps = f["cpsum"].tile([C, 4 * W], F32, tag="cps", name=f"cps{s}_{t}")
        r0 = 4 * t
        for dyi in range(3):
            nc.tensor.matmul(
                ps[:],
                lhsT=wcombr[:, dyi, :],
                rhs=XX3[:, r0 + dyi : r0 + dyi + 4, 0:128],
                start=(dyi == 0),
                stop=False,
            )
        for dyi in range(3):
            nc.tensor.matmul(
                ps[:],
                lhsT=wcombr[:, 3 + dyi, :],
                rhs=XX3[:, r0 + dyi : r0 + dyi + 4, 1:129],
                start=False,
                stop=(dyi == 2),
            )
        out_sb = f["stage"].tile([C, 4, W], F32, tag="stage", name=f"ost{s}_{t}")
        nc.vector.scalar_tensor_tensor(
            out_sb[:],
            ps[:].rearrange("p (a b) -> p a b", b=W),
            b_comb[:],
            XX3[0:64, r0 + 1 : r0 + 5, 1:129].bitcast(F32),
            op0=OP.add,
            op1=OP.add,
        )
        eng = nc.sync if t % 2 == 0 else nc.scalar
        eng.dma_start(out_ap[s, :, r0 : r0 + 4, :], out_sb[:])


def build_program():
    if "nc" in _cache:
        return _cache["nc"]
    nc = bacc.Bacc("TRN2", target_bir_lowering=False, debug=False, enable_asserts=False)
    xs_ap = nc.dram_tensor("xs", [SPB, C, H, W], F32R, kind="ExternalInput").ap()
    wpsA_d = nc.dram_tensor("wpsA", [128, E // 2, 6, C], F32, kind="ExternalInput").ap()
    wpsB_d = nc.dram_tensor("wpsB", [128, E // 2, 6, C], F32, kind="ExternalInput").ap()
    wg1_d = nc.dram_tensor("wg1", [128, GH], F32, kind="ExternalInput").ap()
    bg1_d = nc.dram_tensor("bg1", [GH, 1], F32, kind="ExternalInput").ap()
    wg2_d = nc.dram_tensor("wg2", [GH, E], F32, kind="ExternalInput").ap()
    bg2_d = nc.dram_tensor("bg2", [1, E], F32, kind="ExternalInput").ap()
    bexp_d = nc.dram_tensor("b_exp", [E, C], F32, kind="ExternalInput").ap()
    out_ap = nc.dram_tensor("out", [SPB, C, H, W], F32, kind="ExternalOutput").ap()

    with tile.TileContext(nc) as tc, ExitStack() as ctx:
        pools = {
            "const": ctx.enter_context(tc.tile_pool(name="const", bufs=1)),
            "xx": ctx.enter_context(tc.tile_pool(name="xx", bufs=SPB)),
            "gate": ctx.enter_context(tc.tile_pool(name="gate", bufs=2)),
            "wcomb": ctx.enter_context(tc.tile_pool(name="wcomb", bufs=2)),
            "stage": ctx.enter_context(tc.tile_pool(name="stage", bufs=6)),
            "scratch": ctx.enter_context(tc.tile_pool(name="scratch", bufs=1)),
            "gpsum": ctx.enter_context(tc.tile_pool(name="gpsum", bufs=2, space="PSUM")),
            "cpsum": ctx.enter_context(tc.tile_pool(name="cpsum", bufs=6, space="PSUM")),
        }
        cp = pools["const"]
        # XX tiles + their border memsets first on gpsimd, so the memsets
        # never delay the x-chunk DMAs whose regions they border
        XX0 = pools["xx"].tile([128, FLAT], F32R, tag="XX", name="XX0")
        XX1 = pools["xx"].tile([128, FLAT], F32R, tag="XX", name="XX1")
        _emit_borders(nc, XX0)
        _emit_borders(nc, XX1)
        ones = cp.tile([1, 128], F32)
        nc.gpsimd.memset(ones[:], 1.0)
        # prewarm the ACT exp table before the ACT lane fills with DMAs
        warm = cp.tile([1, 1], F32)
        nc.scalar.activation(warm[:], ones[:, 0:1], ACTF.Exp)
        # tiny gate weights + expert weights B on the gpsimd SWDGE lane
        wg1x2_sb = cp.tile([128, GH], F32)
        nc.gpsimd.dma_start(wg1x2_sb[:], wg1_d[:])
        bg1_sb = cp.tile([GH, 1], F32)
        nc.gpsimd.dma_start(bg1_sb[:], bg1_d[:])
        wg2_sb = cp.tile([GH, E], F32)
        nc.gpsimd.dma_start(wg2_sb[:], wg2_d[:])
        bg2_sb = cp.tile([1, E], F32)
        nc.gpsimd.dma_start(bg2_sb[:], bg2_d[:])
        bexp_sb = cp.tile([E, C], F32)
        nc.gpsimd.dma_start(bexp_sb[:], bexp_d[:])
        wpsA_sb = cp.tile([128, E // 2, 6, C], F32)
        wpsB_sb = cp.tile([128, E // 2, 6, C], F32)
        nc.gpsimd.dma_start(wpsB_sb[:], wpsB_d[:])

        def load_wpsA():
            nc.sync.dma_start(wpsA_sb[:], wpsA_d[:])

        pooled0, bots0 = _emit_sample_loads(nc, pools, 0, XX0, xs_ap, mid_sp=load_wpsA)
        consts = (wg1x2_sb, bg1_sb, wg2_sb, bg2_sb, bexp_sb, wpsA_sb, wpsB_sb, ones)

        g0 = _emit_sample_gate(nc, pools, 0, pooled0, consts)
        # keep s0's exp ahead of the late bottom chunks on the ACT lane
        add_dep_helper(bots0[0].ins, g0[2].ins, sync=False,
                       reason="s0 late bottom DMAs after s0 softmax exp")
        pooled1, bots1 = _emit_sample_loads(nc, pools, 1, XX1, xs_ap)

        _emit_conv_tiles(nc, pools, 0, XX0, *g0[:2], out_ap, range(0, GATE_SPLIT))
        g1 = _emit_sample_gate(nc, pools, 1, pooled1, consts)
        add_dep_helper(bots1[0].ins, g1[2].ins, sync=False,
                       reason="s1 late bottom DMAs after s1 softmax exp")
        _emit_conv_tiles(nc, pools, 0, XX0, *g0[:2], out_ap, range(GATE_SPLIT, NT))
        _emit_conv_tiles(nc, pools, 1, XX1, *g1[:2], out_ap, range(0, NT))

    nc.compile()
    _cache["nc"] = nc
    return nc


def _round_fp32r(a):
    """Round fp32 array to the fp32r grid (RNE to 11-bit mantissa, low 12
    bits of the fp32 word zeroed) — what the PE consumes in fp32r mode."""
    u = np.ascontiguousarray(a, dtype=np.float32).view(np.uint32)
    r = (u + np.uint32(0x7FF) + ((u >> np.uint32(12)) & np.uint32(1))) & np.uint32(
        0xFFFFF000
    )
    return r.view(np.float32)


def host_prep(x, wg1, bg1, wg2, bg2, w_exp, b_exp):
    """Host-side layout prep + per-core sharding. Returns in_maps list."""
    x = _round_fp32r(np.asarray(x, dtype=np.float32))
    wg1 = np.asarray(wg1, dtype=np.float32)
    bg1 = np.asarray(bg1, dtype=np.float32).reshape(GH, 1)
    wg2 = np.asarray(wg2, dtype=np.float32)
    bg2 = np.asarray(bg2, dtype=np.float32).reshape(1, E)
    w_exp = np.asarray(w_exp, dtype=np.float32)
    b_exp = np.asarray(b_exp, dtype=np.float32)

    # w_exp [E, O, I, KH, KW] -> wt [I, E, KH, KW, O]
    wt = np.transpose(w_exp, (2, 0, 3, 4, 1))
    # paired taps: top partitions = dx=-1, bottom = dx=+1
    wpair = np.concatenate([wt[:, :, :, 0, :], wt[:, :, :, 2, :]], axis=0)
    # single taps: dx=0 on top, zeros on bottom
    wsing = np.concatenate([wt[:, :, :, 1, :], np.zeros_like(wt[:, :, :, 1, :])], axis=0)
    # merged [128, E, 6, O]: slots 0-2 pairs, 3-5 singles
    wps = np.concatenate([wpair, wsing], axis=2)

    shared = {
        "wpsA": np.ascontiguousarray(wps[:, 0:4]),
        "wpsB": np.ascontiguousarray(wps[:, 4:8]),
        "wg1": np.ascontiguousarray(np.concatenate([wg1, wg1], axis=0)),
        "bg1": np.ascontiguousarray(bg1),
        "wg2": np.ascontiguousarray(wg2),
        "bg2": np.ascontiguousarray(bg2),
        "b_exp": np.ascontiguousarray(b_exp),
    }
    return [
        {"xs": np.ascontiguousarray(x[SPB * k : SPB * (k + 1)]), **shared}
        for k in range(NCORES)
    ]


def kernel(x, wg1, bg1, wg2, bg2, w_exp, b_exp):
    nc = build_program()
    in_maps = host_prep(x, wg1, bg1, wg2, bg2, w_exp, b_exp)
    res = run_bass_kernel_spmd(nc, in_maps, list(range(NCORES)))
    return np.concatenate([res.results[k]["out"] for k in range(NCORES)], axis=0)
